# revision 5
# baseline (speedup 1.0000x reference)
"""Trainium2 Bass kernel for nn_AdaptiveCoFusion (B=8, L=128, R=49, D=768).

Strategy: pure data parallel — one batch element per NeuronCore (8 cores).
All weights replicated per core, host-packed to bf16 in the exact SBUF
layout so each weight is one contiguous DMA.

Per-core compute keeps activations TRANSPOSED (D on partitions, L on free):
  (x @ W)^T accumulates over 6 K-chunks with W chunks as the stationary
  operand; per-partition biases + tanh fuse into the ScalarE PSUM->SBUF
  copy. The additive-attention score vectors u, v are computed as
  K-chunked matmuls against packed (128,2) vector columns, and the
  broadcast sum u[i] + v[j] is ONE K=2 matmul (lhsT=[ones;u], rhs=[v;ones]).
  Sigmoids are computed as 0.5*tanh(0.5x)+0.5 so ScalarE never leaves the
  exp/tanh table set. The FiltrationGate projections fold on host:
  (txt@Wft)@wfg_t == txt@(Wft@wfg_t), so Wft/Wfm are never shipped.
  Final output is computed in natural orientation (lhsT = transposed
  activations, rhs = Wout chunks) and DMA'd straight from PSUM.
"""

import os
import numpy as np
import ml_dtypes

B, L, R, D = 8, 128, 49, 768
KC = D // 128  # 6
BF_NP = ml_dtypes.bfloat16

LAST = None  # BassKernelResults of the most recent run (for test harness)
_CACHE = {}


def _pack_w(w):
    # (768, 768) -> (128, KC*768): [p, kc*768 + n] = w[kc*128 + p, n]
    return np.ascontiguousarray(
        w.reshape(KC, 128, D).transpose(1, 0, 2).reshape(128, KC * D)
    ).astype(BF_NP)


def _pack_col(v):
    # (768,) -> (128, KC): [p, kc] = v[kc*128 + p]
    return np.ascontiguousarray(v.reshape(KC, 128).T)


def _build():
    from contextlib import ExitStack
    import concourse.bass as bass  # noqa: F401
    import concourse.tile as tile
    from concourse import bacc, mybir
    from concourse.alu_op_type import AluOpType
    from concourse.masks import make_identity

    F32 = mybir.dt.float32
    BF = mybir.dt.bfloat16
    AF = mybir.ActivationFunctionType

    nc = bacc.Bacc("TRN2", target_bir_lowering=False, debug=False,
                   enable_asserts=False)

    txt_d = nc.dram_tensor("txt", [L, D], F32, kind="ExternalInput").ap()
    vis_d = nc.dram_tensor("vis", [R, D], F32, kind="ExternalInput").ap()
    w_d = [nc.dram_tensor(f"w{i}", [128, KC * D], BF, kind="ExternalInput").ap()
           for i in range(9)]
    bias_d = nc.dram_tensor("biasp", [128, 33], F32, kind="ExternalInput").ap()
    vrow_d = nc.dram_tensor("vrow", [128, KC, 8], BF, kind="ExternalInput").ap()
    brow_d = nc.dram_tensor("brow", [1, D], BF, kind="ExternalInput").ap()
    out_d = nc.dram_tensor("out", [L, D], F32, kind="ExternalOutput").ap()

    # bias pack column indices
    C_BT1, C_BI2, C_BGT, C_BGI, C_BRV, C_BA1, C_BA2, C_SFH = 0, 6, 12, 18, 24, 30, 31, 32
    # vrow last-dim columns: 0=wa1_t, 1=wa1_i, 2=wa2_i, 3=wa2_t,
    # 4=wg_i, 5=wg_t, 6=c_t, 7=c_m

    with tile.TileContext(nc) as tc, ExitStack() as ctx:
        const = ctx.enter_context(tc.tile_pool(name="const", bufs=1))
        wpool = ctx.enter_context(tc.tile_pool(name="wpool", bufs=1))
        acts = ctx.enter_context(tc.tile_pool(name="acts", bufs=1))
        tmp = ctx.enter_context(tc.tile_pool(name="tmp", bufs=3))
        psum = ctx.enter_context(tc.tile_pool(name="psum", bufs=2, space="PSUM"))

        # ---- input DMAs (HWDGE ring, FIFO: issue in consumption order)
        txt_f = const.tile([L, D], F32, tag="txtf")
        nc.sync.dma_start(out=txt_f, in_=txt_d)
        vis_f = const.tile([R, D], F32, tag="visf")
        nc.sync.dma_start(out=vis_f, in_=vis_d)
        bias_sb = const.tile([128, 33], F32, tag="bias")
        nc.sync.dma_start(out=bias_sb, in_=bias_d)
        vrow_sb = const.tile([128, KC, 8], BF, tag="vrow")
        nc.sync.dma_start(out=vrow_sb, in_=vrow_d)
        brow_sb = const.tile([1, D], BF, tag="brow")
        nc.sync.dma_start(out=brow_sb, in_=brow_d)
        w_sb = []
        for i in range(9):
            t = wpool.tile([128, KC * D], BF, tag=f"w{i}")
            nc.sync.dma_start(out=t, in_=w_d[i])
            w_sb.append(t)

        ident = const.tile([128, 128], BF, tag="ident")
        make_identity(nc, ident)
        ones_row = const.tile([1, 128], BF, tag="ones")
        nc.vector.memset(ones_row, 1.0)

        # ---- casts f32 -> bf16
        txt_bf = acts.tile([L, D], BF, tag="txtbf")
        nc.vector.tensor_copy(txt_bf, txt_f)
        vis_bf = acts.tile([R, D], BF, tag="visbf")
        nc.vector.tensor_copy(vis_bf, vis_f)

        # ---- transposes: txt_T (d,l), vis_T (d,r)
        txtT = acts.tile([128, KC * 128], BF, tag="txtT")
        for kc in range(KC):
            ps = psum.tile([128, 128], BF, tag="tp")
            nc.tensor.transpose(ps, txt_bf[:, kc * 128:(kc + 1) * 128], ident)
            nc.vector.tensor_copy(txtT[:, kc * 128:(kc + 1) * 128], ps)
        visT = acts.tile([128, KC * R], BF, tag="visT")
        for kc in range(KC):
            ps = psum.tile([128, 128], BF, tag="tp")
            nc.tensor.transpose(ps[:, 0:R], vis_bf[:, kc * 128:(kc + 1) * 128],
                                ident[0:R, 0:R])
            nc.vector.tensor_copy(visT[:, kc * R:(kc + 1) * R], ps[:, 0:R])

        def xw_t(dst, w, rhs, rhs_w, bias_c0, func):
            """dst[:, mc*rhs_w:] = func((x@W)^T chunk + bias) for all 6 mc.

            w: (128, KC*D) packed weight; rhs: transposed activation
            (128, KC*rhs_w); bias_c0: bias pack start col or None.
            """
            for mc in range(KC):
                ps = psum.tile([128, rhs_w], F32, tag="acc")
                for kc in range(KC):
                    nc.tensor.matmul(
                        ps,
                        lhsT=w[:, kc * D + mc * 128: kc * D + (mc + 1) * 128],
                        rhs=rhs[:, kc * rhs_w:(kc + 1) * rhs_w],
                        start=(kc == 0), stop=(kc == KC - 1),
                    )
                if func is None:
                    nc.vector.tensor_copy(dst[:, mc * rhs_w:(mc + 1) * rhs_w], ps)
                else:
                    bias = 0.0 if bias_c0 is None else \
                        bias_sb[:, bias_c0 + mc: bias_c0 + mc + 1]
                    nc.scalar.activation(
                        out=dst[:, mc * rhs_w:(mc + 1) * rhs_w], in_=ps,
                        func=func, bias=bias)

        def vec_reduce(ps_out, pairs, n, start=True):
            """Accumulate sum_d v[d]*act_T[d, :] into ps_out via KC matmuls
            per (vcol_slice, act) pair. ps_out partitions = lhsT free size."""
            first = start
            npairs = len(pairs)
            for i, (c0, c1, act, aw) in enumerate(pairs):
                for kc in range(KC):
                    nc.tensor.matmul(
                        ps_out,
                        lhsT=vrow_sb[:, kc, c0:c1],
                        rhs=act[:, kc * aw: kc * aw + n],
                        start=(first and i == 0 and kc == 0),
                        stop=(i == npairs - 1 and kc == KC - 1),
                    )

        # ---- y1 = tanh(txt@Wt1 + bt1)^T ; u1 = y1 @ wa1_t  (row 1 of (2,128))
        y1 = acts.tile([128, KC * 128], BF, tag="y1")
        xw_t(y1, w_sb[0], txtT, 128, C_BT1, AF.Tanh)
        ps_u1 = psum.tile([1, 128], F32, tag="row")
        vec_reduce(ps_u1, [(0, 1, y1, 128)], 128)
        su1 = acts.tile([1, 128], BF, tag="su1")
        nc.vector.tensor_copy(su1, ps_u1)

        # ---- yv = tanh(vis@Wi1)^T ; v1 = yv @ wa1_i  (row 0 of (2,49))
        yv = acts.tile([128, KC * R], BF, tag="yv")
        xw_t(yv, w_sb[1], visT, R, None, AF.Tanh)
        ps_v1 = psum.tile([1, R], F32, tag="row")
        vec_reduce(ps_v1, [(1, 2, yv, R)], R)
        sv1 = acts.tile([1, R], BF, tag="sv1")
        nc.vector.tensor_copy(sv1, ps_v1)

        # ---- y3 = tanh(txt@Wt2)^T ; v2 = y3 @ wa2_t (row 0) — txt-only, early
        y3 = acts.tile([128, KC * 128], BF, tag="y3")
        xw_t(y3, w_sb[2], txtT, 128, None, AF.Tanh)
        ps_v2 = psum.tile([1, 128], F32, tag="row")
        vec_reduce(ps_v2, [(3, 4, y3, 128)], 128)
        sv2 = acts.tile([1, 128], BF, tag="sv2")
        nc.vector.tensor_copy(sv2, ps_v2)

        # ---- scores1 = u1[l] + v1[r] (+ba1) ; softmax over r ; probs1^T
        ps_s1 = psum.tile([128, R], F32, tag="acc")
        nc.tensor.matmul(ps_s1, lhsT=ones_row, rhs=sv1, start=True, stop=False)
        nc.tensor.matmul(ps_s1, lhsT=su1, rhs=ones_row[:, 0:R], start=False, stop=True)
        probs1 = acts.tile([128, R], F32, tag="p1")
        nc.scalar.activation(out=probs1, in_=ps_s1, func=AF.Exp,
                             bias=bias_sb[:, C_BA1:C_BA1 + 1])
        rs1 = acts.tile([128, 1], F32, tag="rs1")
        nc.vector.reduce_sum(rs1, probs1, axis=mybir.AxisListType.X)
        rr1 = acts.tile([128, 1], F32, tag="rr1")
        nc.vector.reciprocal(rr1, rs1)
        p1bf = acts.tile([128, R], BF, tag="p1bf")
        nc.vector.tensor_scalar_mul(p1bf, probs1, rr1)
        ps_p1t = psum.tile([R, 128], BF, tag="tp")
        nc.tensor.transpose(ps_p1t, p1bf, ident)
        p1T = acts.tile([R, 128], BF, tag="p1T")
        nc.vector.tensor_copy(p1T, ps_p1t)

        # ---- att_img^T[mc] = (probs1 @ vis)^T : lhsT=vis natural chunk (49,128)
        aimgT = acts.tile([128, KC * 128], BF, tag="aimgT")
        for mc in range(KC):
            ps = psum.tile([128, 128], F32, tag="acc")
            nc.tensor.matmul(ps, lhsT=vis_bf[:, mc * 128:(mc + 1) * 128],
                             rhs=p1T, start=True, stop=True)
            nc.vector.tensor_copy(aimgT[:, mc * 128:(mc + 1) * 128], ps)

        # ---- y2 = tanh(att_img@Wi2 + bi2)^T ; u2 = y2 @ wa2_i (row 1)
        y2 = acts.tile([128, KC * 128], BF, tag="y2")
        xw_t(y2, w_sb[3], aimgT, 128, C_BI2, AF.Tanh)
        ps_u2 = psum.tile([1, 128], F32, tag="row")
        vec_reduce(ps_u2, [(2, 3, y2, 128)], 128)
        su2 = acts.tile([1, 128], BF, tag="su2")
        nc.vector.tensor_copy(su2, ps_u2)

        # ---- scores2 = u2[i] + v2[j] (+ba2) ; softmax over j ; probs2^T
        ps_s2 = psum.tile([128, 128], F32, tag="acc")
        nc.tensor.matmul(ps_s2, lhsT=ones_row, rhs=sv2, start=True, stop=False)
        nc.tensor.matmul(ps_s2, lhsT=su2, rhs=ones_row, start=False, stop=True)
        probs2 = acts.tile([128, 128], F32, tag="p2")
        nc.scalar.activation(out=probs2, in_=ps_s2, func=AF.Exp,
                             bias=bias_sb[:, C_BA2:C_BA2 + 1])
        rs2 = acts.tile([128, 1], F32, tag="rs2")
        nc.vector.reduce_sum(rs2, probs2, axis=mybir.AxisListType.X)
        rr2 = acts.tile([128, 1], F32, tag="rr2")
        nc.vector.reciprocal(rr2, rs2)
        p2bf = acts.tile([128, 128], BF, tag="p2bf")
        nc.vector.tensor_scalar_mul(p2bf, probs2, rr2)
        ps_p2t = psum.tile([128, 128], BF, tag="tp")
        nc.tensor.transpose(ps_p2t, p2bf, ident)
        p2T = acts.tile([128, 128], BF, tag="p2T")
        nc.vector.tensor_copy(p2T, ps_p2t)

        # ---- att_text^T[mc] : lhsT = txt natural chunk (128 j, 128 d)
        atxtT = acts.tile([128, KC * 128], BF, tag="atxtT")
        for mc in range(KC):
            ps = psum.tile([128, 128], F32, tag="acc")
            nc.tensor.matmul(ps, lhsT=txt_bf[:, mc * 128:(mc + 1) * 128],
                             rhs=p2T, start=True, stop=True)
            nc.vector.tensor_copy(atxtT[:, mc * 128:(mc + 1) * 128], ps)

        # ---- GMF
        ni = acts.tile([128, KC * 128], BF, tag="ni")
        xw_t(ni, w_sb[4], aimgT, 128, C_BGI, AF.Tanh)
        nt = acts.tile([128, KC * 128], BF, tag="nt")
        xw_t(nt, w_sb[5], atxtT, 128, C_BGT, AF.Tanh)

        ps_g = psum.tile([1, 128], F32, tag="row")
        vec_reduce(ps_g, [(4, 5, ni, 128), (5, 6, nt, 128)], 128)
        tg = acts.tile([1, 128], F32, tag="tg")
        nc.scalar.activation(out=tg, in_=ps_g, func=AF.Tanh, scale=0.5)
        g_bf = acts.tile([1, 128], BF, tag="gbf")
        nc.vector.tensor_scalar(g_bf, tg, 0.5, 0.5,
                                AluOpType.mult, AluOpType.add)
        ps_gb = psum.tile([128, 128], F32, tag="tp")
        nc.tensor.matmul(ps_gb, lhsT=ones_row, rhs=g_bf, start=True, stop=True)
        gbc = acts.tile([128, 128], BF, tag="gbc")
        nc.vector.tensor_copy(gbc, ps_gb)

        mm = acts.tile([128, KC * 128], BF, tag="mm")
        for mc in range(KC):
            sl = slice(mc * 128, (mc + 1) * 128)
            d_t = tmp.tile([128, 128], BF, tag="tmp")
            nc.vector.tensor_sub(d_t, ni[:, sl], nt[:, sl])
            p_t = tmp.tile([128, 128], BF, tag="tmp")
            nc.vector.tensor_mul(p_t, d_t, gbc)
            nc.vector.tensor_add(mm[:, sl], nt[:, sl], p_t)

        # ---- FiltrationGate (host-folded c_t, c_m)
        ps_f = psum.tile([1, 128], F32, tag="row")
        vec_reduce(ps_f, [(6, 7, txtT, 128), (7, 8, mm, 128)], 128)
        tf = acts.tile([1, 128], F32, tag="tf")
        nc.scalar.activation(out=tf, in_=ps_f, func=AF.Tanh, scale=0.5,
                             bias=bias_sb[0:1, C_SFH:C_SFH + 1])
        f_bf = acts.tile([1, 128], BF, tag="fbf")
        nc.vector.tensor_scalar(f_bf, tf, 0.5, 0.5,
                                AluOpType.mult, AluOpType.add)
        ps_fb = psum.tile([128, 128], F32, tag="tp")
        nc.tensor.matmul(ps_fb, lhsT=ones_row, rhs=f_bf, start=True, stop=True)
        fbc = acts.tile([128, 128], BF, tag="fbc")
        nc.vector.tensor_copy(fbc, ps_fb)

        rv = acts.tile([128, KC * 128], BF, tag="rv")
        xw_t(rv, w_sb[6], mm, 128, C_BRV, AF.Tanh)
        res = acts.tile([128, KC * 128], BF, tag="res")
        for mc in range(KC):
            sl = slice(mc * 128, (mc + 1) * 128)
            nc.vector.tensor_mul(res[:, sl], rv[:, sl], fbc)

        # ---- output (natural orientation) = txt@Wout_t + reserved@Wout_m + bout
        out_sb = acts.tile([L, D], F32, tag="outsb")
        for o0, osz in ((0, 512), (512, 256)):
            ps_o = psum.tile([128, osz], F32, tag="outp")
            for kc in range(KC):
                nc.tensor.matmul(
                    ps_o, lhsT=txtT[:, kc * 128:(kc + 1) * 128],
                    rhs=w_sb[7][:, kc * D + o0: kc * D + o0 + osz],
                    start=(kc == 0), stop=False)
            for kc in range(KC):
                nc.tensor.matmul(
                    ps_o, lhsT=res[:, kc * 128:(kc + 1) * 128],
                    rhs=w_sb[8][:, kc * D + o0: kc * D + o0 + osz],
                    start=False, stop=False)
            nc.tensor.matmul(ps_o, lhsT=ones_row,
                             rhs=brow_sb[:, o0:o0 + osz],
                             start=False, stop=True)
            nc.vector.tensor_copy(out_sb[:, o0:o0 + osz], ps_o)
            nc.sync.dma_start(out=out_d[:, o0:o0 + osz], in_=out_sb[:, o0:o0 + osz])

    nc.compile()
    return nc


def _inputs_pack(inp):
    f32 = np.float32
    g = lambda k: np.asarray(inp[k], dtype=f32)

    ws = [_pack_w(g(k)) for k in
          ("Wt1", "Wi1", "Wt2", "Wi2", "Wgi", "Wgt", "Wrv", "Wout_t", "Wout_m")]

    c_t = (g("Wft").astype(np.float64) @ g("wfg_t").astype(np.float64))
    c_m = (g("Wfm").astype(np.float64) @ g("wfg_m").astype(np.float64))
    s_fh = 0.5 * (float(g("bfm").astype(np.float64) @ g("wfg_m").astype(np.float64))
                  + float(g("bfg")))

    bias = np.zeros((128, 33), np.float32)
    bias[:, 0:6] = _pack_col(g("bt1"))
    bias[:, 6:12] = _pack_col(g("bi2"))
    bias[:, 12:18] = _pack_col(g("bgt"))
    bias[:, 18:24] = _pack_col(g("bgi"))
    bias[:, 24:30] = _pack_col(g("brv"))
    bias[:, 30] = float(g("ba1"))
    bias[:, 31] = float(g("ba2"))
    bias[:, 32] = s_fh

    vrow = np.zeros((128, KC, 8), np.float32)
    vrow[:, :, 0] = _pack_col(g("wa1_t"))
    vrow[:, :, 1] = _pack_col(g("wa1_i"))
    vrow[:, :, 2] = _pack_col(g("wa2_i"))
    vrow[:, :, 3] = _pack_col(g("wa2_t"))
    vrow[:, :, 4] = _pack_col(g("wg_i"))
    vrow[:, :, 5] = _pack_col(g("wg_t"))
    vrow[:, :, 6] = _pack_col(c_t.astype(np.float32))
    vrow[:, :, 7] = _pack_col(c_m.astype(np.float32))
    vrow = vrow.astype(BF_NP)

    brow = np.ascontiguousarray(g("bout").reshape(1, D)).astype(BF_NP)

    shared = {f"w{i}": ws[i] for i in range(9)}
    shared.update(biasp=bias, vrow=vrow, brow=brow)

    txt = g("txt_hidden")
    vis = g("vis_hidden")
    in_maps = []
    for c in range(B):
        m = dict(shared)
        m["txt"] = np.ascontiguousarray(txt[c])
        m["vis"] = np.ascontiguousarray(vis[c])
        in_maps.append(m)
    return in_maps


def kernel(**inputs):
    global LAST
    from concourse import bass_utils

    nc = _CACHE.get("nc")
    if nc is None:
        nc = _build()
        _CACHE["nc"] = nc

    in_maps = _inputs_pack(inputs)
    res = bass_utils.run_bass_kernel_spmd(
        nc, in_maps, core_ids=list(range(B)),
        trace=bool(os.environ.get("KERNEL_TRACE")),
    )
    LAST = res
    out = np.stack([np.asarray(res.results[c]["out"]) for c in range(B)], axis=0)
    return out.astype(np.float32)
